# revision 1
# baseline (speedup 1.0000x reference)
import os, sys
import numpy as np

sys.path.insert(0, "/opt/trn_rl_repo")

from concourse import bass, bacc, bass_utils
from concourse import mybir
from concourse.tile import TileContext

F32 = mybir.dt.float32
F32R = mybir.dt.float32r
F16 = mybir.dt.float16
ALU = mybir.AluOpType
ACTF = mybir.ActivationFunctionType

A = 32          # in_maps
B = 32          # out_maps
C = 16          # atoms
H = 64
W = 64
NCORES = 8
ROWS = H // NCORES
NPOS = ROWS * W             # 512 positions per core
NCHUNK = 128
NCH = NPOS // NCHUNK        # 4 chunks
CB = C * B                  # 512
EPS = 1e-4
G = 14                      # a-values packed per partition-stride group
J = 3                       # ceil(A/G)
NP_IN = 9 * G               # 126 partitions for patch/weight tensors

_CACHE = {}


def _build_nc(num_routes: int):
    nc = bacc.Bacc(None, target_bir_lowering=False)

    patches_h_d = nc.declare_dram_parameter("patches_h", [9, A, NPOS], F32R, isOutput=False)
    patches_l_d = nc.declare_dram_parameter("patches_l", [9, A, NPOS], F32R, isOutput=False)
    wv_h_d = nc.declare_dram_parameter("wv_h", [9, A, CB], F32R, isOutput=False)
    wv_l_d = nc.declare_dram_parameter("wv_l", [9, A, CB], F32R, isOutput=False)
    out_d = nc.declare_dram_parameter("out", [NPOS, CB], F32, isOutput=True)

    with TileContext(nc) as tc:
        with (
            tc.tile_pool(name="const", bufs=1) as cpool,
            tc.tile_pool(name="psum", bufs=6, space="PSUM") as pp,
            tc.tile_pool(name="p1psum", bufs=2, space="PSUM") as pp1,
            tc.tile_pool(name="v1", bufs=1) as v1pool,
            tc.tile_pool(name="v2", bufs=1) as v2pool,
            tc.tile_pool(name="work", bufs=1) as wk,
            tc.tile_pool(name="small", bufs=1) as sm,
        ):

            for k in range(NCH):
                v1 = v1pool.tile([NCHUNK, CB * A], F16)    # [p, (c b a)]
                v2 = v2pool.tile([NCHUNK, CB * A], F32)    # [p, (b a c)]
                p1raw = sm.tile([NCHUNK, CB], F32, tag="p1raw")
                p1ps = pp1.tile([NCHUNK, CB], F32)
                v14 = v1[:].rearrange("p (c b a) -> p c b a", c=C, b=B)
                v24 = v2[:].rearrange("p (b a c) -> p b a c", b=B, a=A)

                pch = cpool.tile([9, A * NCHUNK], F32R, tag="pch")
                pcl = cpool.tile([9, A * NCHUNK], F32R, tag="pcl")
                ksl = slice(k * NCHUNK, (k + 1) * NCHUNK)
                nc.sync.dma_start(out=pch[:].rearrange("q (a n) -> q a n", a=A),
                                  in_=patches_h_d.ap()[:, :, ksl])
                nc.sync.dma_start(out=pcl[:].rearrange("q (a n) -> q a n", a=A),
                                  in_=patches_l_d.ap()[:, :, ksl])
                pch3 = pch[:].rearrange("q (a n) -> q a n", a=A)
                pcl3 = pcl[:].rearrange("q (a n) -> q a n", a=A)
                QA = 8
                for qa in range(A // QA):
                    wqh = cpool.tile([9, QA * CB], F32R, tag="wqh")
                    wql = cpool.tile([9, QA * CB], F32R, tag="wql")
                    asl = slice(qa * QA, (qa + 1) * QA)
                    nc.sync.dma_start(out=wqh[:].rearrange("q (a n) -> q a n", a=QA),
                                      in_=wv_h_d.ap()[:, asl, :])
                    nc.sync.dma_start(out=wql[:].rearrange("q (a n) -> q a n", a=QA),
                                      in_=wv_l_d.ap()[:, asl, :])
                    wqh3 = wqh[:].rearrange("q (a n) -> q a n", a=QA)
                    wql3 = wql[:].rearrange("q (a n) -> q a n", a=QA)
                    for al in range(QA):
                        a = qa * QA + al
                        hi_p = pch3[:, a, :]
                        lo_p = pcl3[:, a, :]
                        hi_w = wqh3[:, al, :]
                        lo_w = wql3[:, al, :]
                        nc.tensor.matmul(out=p1ps[:], lhsT=hi_p, rhs=hi_w,
                                         start=(a == 0), stop=False)
                        nc.tensor.matmul(out=p1ps[:], lhsT=hi_p, rhs=lo_w,
                                         start=False, stop=False)
                        nc.tensor.matmul(out=p1ps[:], lhsT=lo_p, rhs=hi_w,
                                         start=False, stop=(a == A - 1))
                        vps = pp.tile([NCHUNK, CB], F32)
                        nc.tensor.matmul(out=vps[:], lhsT=hi_p, rhs=hi_w,
                                         start=True, stop=False)
                        nc.tensor.matmul(out=vps[:], lhsT=hi_p, rhs=lo_w,
                                         start=False, stop=False)
                        nc.tensor.matmul(out=vps[:], lhsT=lo_p, rhs=hi_w,
                                         start=False, stop=True)
                        srcv = vps[:].rearrange("p (c b) -> p c b", c=C)
                        nc.vector.tensor_copy(out=v24[:, :, a, :],
                                              in_=srcv.rearrange("p c b -> p b c"))
                        nc.gpsimd.tensor_copy(
                            out=v14[:, :, :, a],
                            in_=v24[:, :, a, :].rearrange("p b c -> p c b"))
                nc.vector.tensor_scalar_mul(out=p1raw[:], in0=p1ps[:], scalar1=1.0 / A)

                logits = sm.tile([NCHUNK, B * A], F32, tag="logits")   # [p, (b a)]
                lsm = sm.tile([NCHUNK, B * A], F32, tag="lsm")
                lsm16 = wk.tile([NCHUNK, B * A], F16, tag="lsm16")
                pcur = sm.tile([NCHUNK, CB], F32, tag="pcur")          # [p, (c b)]
                praw = sm.tile([NCHUNK, CB], F32, tag="praw")
                tmpf = wk.tile([NCHUNK, CB * A // 4], F32, tag="tmpf")
                tmph = tmpf[:].bitcast(F16)

                def squash(p_raw):
                    sq = sm.tile([NCHUNK, B], F32, tag="sq")
                    den = sm.tile([NCHUNK, B], F32, tag="den")
                    fac = sm.tile([NCHUNK, B], F32, tag="fac")
                    p2t = wk.tile([NCHUNK, CB], F32, tag="p2")
                    p2 = p2t[:]
                    nc.vector.tensor_tensor(out=p2, in0=p_raw[:], in1=p_raw[:],
                                            op=ALU.mult)
                    nc.vector.tensor_reduce(
                        out=sq[:],
                        in_=p2.rearrange("p (c b) -> p b c", c=C),
                        axis=mybir.AxisListType.X, op=ALU.add)
                    nc.vector.tensor_scalar_add(out=sq[:], in0=sq[:], scalar1=EPS)
                    nc.scalar.activation(out=den[:], in_=sq[:], func=ACTF.Sqrt)
                    nc.vector.tensor_scalar_add(out=fac[:], in0=sq[:], scalar1=1.0)
                    nc.vector.tensor_tensor(out=den[:], in0=den[:], in1=fac[:],
                                            op=ALU.mult)
                    nc.vector.tensor_scalar_add(out=den[:], in0=den[:], scalar1=EPS)
                    nc.vector.reciprocal(out=den[:], in_=den[:])
                    nc.vector.tensor_tensor(out=fac[:], in0=sq[:], in1=den[:],
                                            op=ALU.mult)
                    nc.vector.tensor_tensor(
                        out=pcur[:].rearrange("p (c b) -> p c b", c=C),
                        in0=p_raw[:].rearrange("p (c b) -> p c b", c=C),
                        in1=fac[:].unsqueeze(1).to_broadcast([NCHUNK, C, B]),
                        op=ALU.mult)

                def delta_update(first):
                    # tmpf[p,b,a,c] = V2 * pcur (bcast a); reduce c -> delta [p,(b a)]
                    QB = B // 4
                    pc_bac = pcur[:].rearrange("p (c b) -> p b c", c=C)
                    for h in range(4):
                        bs = slice(h * QB, (h + 1) * QB)
                        nc.vector.tensor_tensor(
                            out=tmpf[:].rearrange("p (b a c) -> p b a c", b=QB, a=A),
                            in0=v24[:, bs, :, :],
                            in1=pc_bac[:, bs, :].unsqueeze(2)
                                .to_broadcast([NCHUNK, QB, A, C]),
                            op=ALU.mult)
                        nc.vector.tensor_reduce(
                            out=logits[:].rearrange("p (b a) -> p b a", b=B)[:, bs, :],
                            in_=tmpf[:].rearrange("p (b a c) -> p b a c", b=QB, a=A),
                            axis=mybir.AxisListType.X, op=ALU.add)
                    if first:
                        nc.vector.tensor_scalar_add(out=logits[:], in0=logits[:],
                                                    scalar1=1.0 / A)
                    else:
                        nc.vector.tensor_tensor(out=logits[:], in0=logits[:],
                                                in1=lsm[:], op=ALU.add)

                def softmax():
                    ssum = sm.tile([NCHUNK, B], F32, tag="ssum")
                    nc.scalar.activation(out=lsm[:], in_=logits[:], func=ACTF.Exp)
                    nc.vector.tensor_reduce(
                        out=ssum[:], in_=lsm[:].rearrange("p (b a) -> p b a", b=B),
                        axis=mybir.AxisListType.X, op=ALU.add)
                    nc.vector.reciprocal(out=ssum[:], in_=ssum[:])
                    nc.vector.tensor_tensor(
                        out=lsm[:].rearrange("p (b a) -> p b a", b=B),
                        in0=lsm[:].rearrange("p (b a) -> p b a", b=B),
                        in1=ssum[:].unsqueeze(2).to_broadcast([NCHUNK, B, A]),
                        op=ALU.mult)
                    nc.gpsimd.tensor_copy(out=lsm16[:], in_=lsm[:])

                def preds_from_lsm():
                    HC = C // 2
                    for h in range(2):
                        cs = slice(h * HC, (h + 1) * HC)
                        nc.vector.tensor_tensor(
                            out=tmph.rearrange("p (c b a) -> p c b a", c=HC, b=B),
                            in0=v14[:, cs, :, :],
                            in1=lsm16[:].rearrange("p (b a) -> p b a", b=B)
                                .unsqueeze(1).to_broadcast([NCHUNK, HC, B, A]),
                            op=ALU.mult)
                        nc.vector.tensor_reduce(
                            out=praw[:].rearrange("p (c b) -> p c b", c=C)[:, cs, :],
                            in_=tmph.rearrange("p (c b a) -> p c b a", c=HC, b=B),
                            axis=mybir.AxisListType.X, op=ALU.add)

                squash(p1raw)
                if num_routes >= 2:
                    delta_update(True)
                    for it in range(2, num_routes + 1):
                        softmax()
                        preds_from_lsm()
                        squash(praw)
                        if it < num_routes:
                            delta_update(False)

                nc.sync.dma_start(out=out_d.ap()[k * NCHUNK:(k + 1) * NCHUNK, :],
                                  in_=pcur[:])

    nc.compile()
    return nc


def kernel(x=None, weights=None, num_routes=3, **kw):
    x = np.asarray(x, dtype=np.float32)
    weights = np.asarray(weights, dtype=np.float32)
    nr = int(num_routes)

    if nr not in _CACHE:
        _CACHE[nr] = _build_nc(nr)
    nc = _CACHE[nr]

    xp = np.zeros((A, H + 2, W + 2), dtype=np.float32)
    xp[:, 1:-1, 1:-1] = x

    f16r = lambda t: t.astype(np.float16).astype(np.float32)
    wvf = np.ascontiguousarray(weights.reshape(9, A, CB))
    wv_hi = f16r(wvf)
    wv_lo = wvf - wv_hi

    in_maps = []
    for core in range(NCORES):
        r0 = core * ROWS
        pat = np.empty((9, A, ROWS, W), dtype=np.float32)
        for dp in range(3):
            for dq in range(3):
                pat[dp * 3 + dq] = xp[:, r0 + dp:r0 + dp + ROWS, dq:dq + W]
        patf = np.ascontiguousarray(pat.reshape(9, A, NPOS))
        pat_hi = f16r(patf)
        in_maps.append({"patches_h": pat_hi, "patches_l": patf - pat_hi,
                        "wv_h": wv_hi, "wv_l": wv_lo})

    res = bass_utils.run_bass_kernel_spmd(nc, in_maps, core_ids=list(range(NCORES)))

    out = np.empty((B, C, H, W), dtype=np.float32)
    for core in range(NCORES):
        o = np.asarray(res.results[core]["out"]).reshape(ROWS, W, C, B)
        out[:, :, core * ROWS:(core + 1) * ROWS, :] = o.transpose(3, 2, 0, 1)
    return out


def profile_once(inputs):
    """Run once with NTFF tracing on core 0 and return HW exec time in ns."""
    x = np.asarray(inputs["x"], dtype=np.float32)
    weights = np.asarray(inputs["weights"], dtype=np.float32)
    nr = int(inputs.get("num_routes", 3))
    if nr not in _CACHE:
        _CACHE[nr] = _build_nc(nr)
    nc = _CACHE[nr]
    xp = np.zeros((A, H + 2, W + 2), dtype=np.float32)
    xp[:, 1:-1, 1:-1] = x
    f16r = lambda t: t.astype(np.float16).astype(np.float32)
    wvf = np.ascontiguousarray(weights.reshape(9, A, CB))
    wv_hi = f16r(wvf); wv_lo = wvf - wv_hi
    in_maps = []
    for core in range(NCORES):
        r0 = core * ROWS
        pat = np.empty((9, A, ROWS, W), dtype=np.float32)
        for dp in range(3):
            for dq in range(3):
                pat[dp * 3 + dq] = xp[:, r0 + dp:r0 + dp + ROWS, dq:dq + W]
        patf = np.ascontiguousarray(pat.reshape(9, A, NPOS))
        pat_hi = f16r(patf)
        in_maps.append({"patches_h": pat_hi, "patches_l": patf - pat_hi,
                        "wv_h": wv_hi, "wv_l": wv_lo})
    res = bass_utils.run_bass_kernel_spmd(nc, in_maps,
                                          core_ids=list(range(NCORES)),
                                          trace=True, trace_cores=[0])
    if res.exec_time_ns is not None:
        return int(res.exec_time_ns)
    raise RuntimeError("no exec_time_ns from trace")



# revision 2
# speedup vs baseline: 2.7317x; 2.7317x over previous
import os, sys
import numpy as np

sys.path.insert(0, "/opt/trn_rl_repo")

from concourse import bass, bacc, bass_utils
from concourse import mybir
from concourse.tile import TileContext

F32 = mybir.dt.float32
F16 = mybir.dt.float16
ALU = mybir.AluOpType
ACTF = mybir.ActivationFunctionType

A = 32          # in_maps
B = 32          # out_maps
C = 16          # atoms
H = 64
W = 64
NCORES = 8
ROWS = H // NCORES
NPOS = ROWS * W             # 512 positions per core
NCHUNK = 128
NCH = NPOS // NCHUNK        # 4 chunks
CB = C * B                  # 512
K27 = 27                    # stacked hi/lo contraction rows (9*3)
NJ = 7                      # ceil(27*A/128) p1 slabs
EPS = 1e-4
GA = 4                      # a-values per PSUM group
NG = A // GA                # 8 groups
QC = 4                      # c-values per preds/delta2 quarter
NQ = C // QC                # 4 quarters

# delta1 tree engine pattern: True -> Pool, False -> DVE (per group)
D1_TREE_POOL = [True] * 8
PREDS_POOL = [False, True, False, True]
PREDS_POOL2 = [True, False, False, True]
D2_POOL = [False, True, False, True]

_CACHE = {}


def _build_nc(num_routes: int):
    nc = bacc.Bacc(None, target_bir_lowering=False)

    pstack_d = nc.declare_dram_parameter("pstack", [K27, A, NPOS], F16, isOutput=False)
    wstack_d = nc.declare_dram_parameter("wstack", [K27, A, CB], F16, isOutput=False)
    p1p_d = nc.declare_dram_parameter("p1p", [128, NJ, NPOS], F16, isOutput=False)
    p1w_d = nc.declare_dram_parameter("p1w", [128, NJ, CB], F16, isOutput=False)
    out_d = nc.declare_dram_parameter("out", [NPOS, CB], F16, isOutput=True)

    with TileContext(nc) as tc:
        with (
            tc.tile_pool(name="const", bufs=1) as cpool,
            tc.tile_pool(name="pch", bufs=2) as ppool,
            tc.tile_pool(name="psum", bufs=2, space="PSUM") as pp,
            tc.tile_pool(name="v16", bufs=2) as vpool,
            tc.tile_pool(name="t16", bufs=2) as tpool,
            tc.tile_pool(name="t32", bufs=2) as t32pool,
            tc.tile_pool(name="small", bufs=2) as sm,
        ):
            p1w = cpool.tile([128, NJ * CB], F16, tag="p1w")
            nc.sync.dma_start(out=p1w[:].rearrange("p (j n) -> p j n", j=NJ),
                              in_=p1w_d.ap())
            p1w3 = p1w[:].rearrange("p (j n) -> p j n", j=NJ)
            p1pa = cpool.tile([128, NJ * NPOS], F16, tag="p1pa")
            nc.sync.dma_start(out=p1pa[:].rearrange("p (j n) -> p j n", j=NJ),
                              in_=p1p_d.ap())
            p1pa3 = p1pa[:].rearrange("p (j n) -> p j n", j=NJ)
            wst = cpool.tile([K27, A * CB], F16, tag="wst")
            nc.sync.dma_start(out=wst[:].rearrange("q (a n) -> q a n", a=A),
                              in_=wstack_d.ap())
            wst3 = wst[:].rearrange("q (a n) -> q a n", a=A)
            biasE = cpool.tile([128, 1], F32, tag="biasE")
            nc.vector.memset(biasE[:], EPS)

            def make_chunk(k):
                ksl = slice(k * NCHUNK, (k + 1) * NCHUNK)

                pch = ppool.tile([K27, A * NCHUNK], F16, tag="pch")
                nc.sync.dma_start(out=pch[:].rearrange("q (a n) -> q a n", a=A),
                                  in_=pstack_d.ap()[:, :, ksl])
                pch3 = pch[:].rearrange("q (a n) -> q a n", a=A)
                p1p3 = p1pa3[:, :, ksl]

                # v16: [p, a, (c b)] f16 — contiguous drains, mid-broadcast mults
                v16 = vpool.tile([NCHUNK, A * CB], F16, tag="v16")
                vA = v16[:].rearrange("p (a n) -> p a n", a=A)
                vACB = v16[:].rearrange("p (a c b) -> p a c b", a=A, c=C)

                # p1 (mean over a) via stacked matmuls, first
                p1ps_t = pp.tile([NCHUNK, GA * CB], F32, tag="vps")
                p1ps = p1ps_t[:][:, :CB]
                for j in range(NJ):
                    nc.tensor.matmul(out=p1ps, lhsT=p1p3[:, j, :],
                                     rhs=p1w3[:, j, :],
                                     start=(j == 0), stop=(j == NJ - 1))

                logits = sm.tile([NCHUNK, A * B], F32, tag="logits")  # [p,(a b)]
                lg3 = logits[:].rearrange("p (a b) -> p a b", a=A)
                lsm16 = sm.tile([NCHUNK, A * B], F16, tag="lsm16")    # [p,(a b)]
                lsm3 = lsm16[:].rearrange("p (a b) -> p a b", a=A)
                e32 = sm.tile([NCHUNK, A * B], F32, tag="e32")
                e3 = e32[:].rearrange("p (a b) -> p a b", a=A)
                ssum = sm.tile([NCHUNK, B], F32, tag="ssum")
                praw1 = sm.tile([NCHUNK, CB], F32, tag="praw1")
                pcur1 = sm.tile([NCHUNK, CB], F32, tag="pcur1")
                praw16 = sm.tile([NCHUNK, CB], F16, tag="praw16")
                pcur16 = sm.tile([NCHUNK, CB], F16, tag="pcur16")
                p2 = sm.tile([NCHUNK, CB], F32, tag="p2")
                p216 = p2[:].bitcast(F16)[:, :CB]
                d2a = sm.tile([NCHUNK, A * B], F16, tag="d2a")        # [p,(a b)]
                d3 = d2a[:].rearrange("p (a b) -> p a b", a=A)
                sq = sm.tile([NCHUNK, B], F32, tag="sq")
                den = sm.tile([NCHUNK, B], F32, tag="den")
                fac = sm.tile([NCHUNK, B], F32, tag="fac")
                fac16 = sm.tile([NCHUNK, B], F16, tag="fac16")

                def squash_tail(praw_ap, pcur_ap, f16mode):
                    nc.scalar.activation(out=den[:], in_=sq[:], func=ACTF.Sqrt,
                                         bias=biasE[:])
                    nc.vector.tensor_scalar_add(out=fac[:], in0=sq[:],
                                                scalar1=1.0 + EPS)
                    nc.vector.tensor_tensor(out=den[:], in0=den[:], in1=fac[:],
                                            op=ALU.mult)
                    nc.vector.tensor_scalar_add(out=den[:], in0=den[:], scalar1=EPS)
                    nc.vector.reciprocal(out=den[:], in_=den[:])
                    nc.vector.scalar_tensor_tensor(out=fac[:], in0=sq[:],
                                                   scalar=EPS, in1=den[:],
                                                   op0=ALU.add, op1=ALU.mult)
                    if f16mode:
                        nc.vector.tensor_copy(out=fac16[:], in_=fac[:])
                        fin = fac16[:].unsqueeze(1).to_broadcast([NCHUNK, C, B])
                    else:
                        fin = fac[:].unsqueeze(1).to_broadcast([NCHUNK, C, B])
                    nc.vector.tensor_tensor(
                        out=pcur_ap.rearrange("p (c b) -> p c b", c=C),
                        in0=praw_ap.rearrange("p (c b) -> p c b", c=C),
                        in1=fin, op=ALU.mult)

                def squash1():
                    nc.scalar.activation(out=praw1[:], in_=p1ps, func=ACTF.Copy,
                                         scale=1.0 / A)
                    nc.vector.tensor_tensor(out=p2[:], in0=praw1[:], in1=praw1[:],
                                            op=ALU.mult)
                    nc.vector.tensor_reduce(
                        out=sq[:], in_=p2[:].rearrange("p (c b) -> p b c", c=C),
                        axis=mybir.AxisListType.X, op=ALU.add)
                    squash_tail(praw1[:], pcur1[:], False)

                def squash23():
                    nc.vector.tensor_tensor(out=p216, in0=praw16[:],
                                            in1=praw16[:], op=ALU.mult)
                    nc.vector.tensor_reduce(
                        out=sq[:], in_=p216.rearrange("p (c b) -> p b c", c=C),
                        axis=mybir.AxisListType.X, op=ALU.add)
                    squash_tail(praw16[:], pcur16[:], True)

                def delta1():
                    # fused: votes matmuls + contiguous v16 drain + delta1
                    pcb = pcur1[:].unsqueeze(1).to_broadcast([NCHUNK, GA, CB])
                    for g in range(NG):
                        vps = pp.tile([NCHUNK, GA * CB], F32, tag="vps")
                        vps3 = vps[:].rearrange("p (j n) -> p j n", j=GA)
                        for j in range(GA):
                            a = g * GA + j
                            nc.tensor.matmul(out=vps3[:, j, :],
                                             lhsT=pch3[:, a, :], rhs=wst3[:, a, :],
                                             start=True, stop=True)
                        # contiguous drain (Act-legal)
                        nc.scalar.activation(
                            out=vA[:, g * GA:(g + 1) * GA, :],
                            in_=vps3, func=ACTF.Copy)
                        # delta1 mult on DVE (only engine that may read PSUM)
                        t32 = t32pool.tile([NCHUNK, GA * CB], F32)
                        t4 = t32[:].rearrange("p (j c b) -> p j c b", j=GA, c=C)
                        lgout = lg3[:, g * GA:(g + 1) * GA, :]
                        nc.vector.tensor_tensor(
                            out=t32[:].rearrange("p (j n) -> p j n", j=GA),
                            in0=vps3, in1=pcb, op=ALU.mult)
                        eng = nc.gpsimd if D1_TREE_POOL[g] else nc.vector
                        eng.tensor_tensor(
                            out=t4[:, :, 0:8, :], in0=t4[:, :, 0:8, :],
                            in1=t4[:, :, 8:16, :], op=ALU.add)
                        eng.tensor_tensor(
                            out=t4[:, :, 0:4, :], in0=t4[:, :, 0:4, :],
                            in1=t4[:, :, 4:8, :], op=ALU.add)
                        eng.tensor_tensor(
                            out=t4[:, :, 0:2, :], in0=t4[:, :, 0:2, :],
                            in1=t4[:, :, 2:4, :], op=ALU.add)
                        eng.tensor_tensor(
                            out=lgout, in0=t4[:, :, 0, :],
                            in1=t4[:, :, 1, :], op=ALU.add)

                def softmax(it):
                    src = logits[:] if it == 2 else d2a[:]
                    srcv = lg3 if it == 2 else d3
                    nc.scalar.activation(out=e32[:], in_=src, func=ACTF.Exp)
                    nc.vector.tensor_reduce(
                        out=ssum[:],
                        in_=e32[:].rearrange("p (a b) -> p b a", a=A),
                        axis=mybir.AxisListType.X, op=ALU.add)
                    nc.vector.reciprocal(out=ssum[:], in_=ssum[:])
                    nc.gpsimd.tensor_tensor(
                        out=lsm3, in0=e3,
                        in1=ssum[:].unsqueeze(1).to_broadcast([NCHUNK, A, B]),
                        op=ALU.mult)

                def preds(it):
                    # praw16[p,(c b)] = sum_a v16 * lsm16 (f16, c-quarters)
                    pat = PREDS_POOL if it == 2 else PREDS_POOL2
                    lb = lsm3.unsqueeze(2).to_broadcast([NCHUNK, A, QC, B])
                    for q in range(NQ):
                        eng = nc.gpsimd if pat[q] else nc.vector
                        t16 = tpool.tile([NCHUNK, A * QC * B], F16)
                        t4 = t16[:].rearrange("p (a c b) -> p a c b", a=A, c=QC)
                        eng.tensor_tensor(
                            out=t4, in0=vACB[:, :, q * QC:(q + 1) * QC, :],
                            in1=lb, op=ALU.mult)
                        eng.tensor_tensor(out=t4[:, 0:16, :, :],
                                          in0=t4[:, 0:16, :, :],
                                          in1=t4[:, 16:32, :, :], op=ALU.add)
                        eng.tensor_tensor(out=t4[:, 0:8, :, :],
                                          in0=t4[:, 0:8, :, :],
                                          in1=t4[:, 8:16, :, :], op=ALU.add)
                        eng.tensor_tensor(out=t4[:, 0:4, :, :],
                                          in0=t4[:, 0:4, :, :],
                                          in1=t4[:, 4:8, :, :], op=ALU.add)
                        eng.tensor_tensor(out=t4[:, 0:2, :, :],
                                          in0=t4[:, 0:2, :, :],
                                          in1=t4[:, 2:4, :, :], op=ALU.add)
                        eng.tensor_tensor(
                            out=praw16[:][:, q * QC * B:(q + 1) * QC * B],
                            in0=t4[:, 0, :, :], in1=t4[:, 1, :, :], op=ALU.add)

                def delta2():
                    # d2a[p,(a b)] = sum_c v16 * pcur16 + lsm16 (f16 2x)
                    parts = []
                    for q in range(NQ):
                        eng = nc.gpsimd if D2_POOL[q] else nc.vector
                        t16 = tpool.tile([NCHUNK, A * QC * B], F16)
                        t4 = t16[:].rearrange("p (a c b) -> p a c b", a=A, c=QC)
                        pcb = pcur16[:].rearrange("p (c b) -> p c b", c=C) \
                            [:, q * QC:(q + 1) * QC, :].unsqueeze(1) \
                            .to_broadcast([NCHUNK, A, QC, B])
                        eng.tensor_tensor(
                            out=t4, in0=vACB[:, :, q * QC:(q + 1) * QC, :],
                            in1=pcb, op=ALU.mult)
                        eng.tensor_tensor(out=t4[:, :, 0:2, :],
                                          in0=t4[:, :, 0:2, :],
                                          in1=t4[:, :, 2:4, :], op=ALU.add)
                        eng.tensor_tensor(out=t4[:, :, 0, :],
                                          in0=t4[:, :, 0, :],
                                          in1=t4[:, :, 1, :], op=ALU.add)
                        parts.append(t4[:, :, 0, :])
                    nc.vector.tensor_tensor(out=d3, in0=parts[0], in1=parts[1],
                                            op=ALU.add)
                    nc.gpsimd.tensor_tensor(out=parts[2], in0=parts[2],
                                            in1=parts[3], op=ALU.add)
                    nc.vector.tensor_tensor(out=d3, in0=d3, in1=parts[2],
                                            op=ALU.add)
                    nc.vector.tensor_tensor(out=d2a[:], in0=d2a[:], in1=lsm16[:],
                                            op=ALU.add)

                def phaseA():
                    squash1()
                    if num_routes >= 2:
                        delta1()

                def phaseB():
                    if num_routes == 1:
                        nc.vector.tensor_copy(out=pcur16[:], in_=pcur1[:])
                    else:
                        for it in range(2, num_routes + 1):
                            if it > 2:
                                delta2()
                            softmax(it)
                            preds(it)
                            squash23()
                    nc.sync.dma_start(
                        out=out_d.ap()[k * NCHUNK:(k + 1) * NCHUNK, :],
                        in_=pcur16[:])
                return phaseA, phaseB

            for k in range(NCH):
                pA, pB = make_chunk(k)
                pA()
                pB()

    nc.compile()
    return nc


def _prep_inputs(x, weights):
    xp = np.zeros((A, H + 2, W + 2), dtype=np.float32)
    xp[:, 1:-1, 1:-1] = x
    f16 = np.float16

    wv = np.ascontiguousarray(weights.reshape(9, A, CB)).astype(np.float32)
    wh = wv.astype(f16).astype(np.float32)
    wl = (wv - wh).astype(f16)
    wstack = np.empty((K27, A, CB), dtype=f16)
    wstack[0:9] = wh.astype(f16)
    wstack[9:18] = wh.astype(f16)
    wstack[18:27] = wl

    per_core = []
    for core in range(NCORES):
        r0 = core * ROWS
        pat = np.empty((9, A, ROWS, W), dtype=np.float32)
        for dp in range(3):
            for dq in range(3):
                pat[dp * 3 + dq] = xp[:, r0 + dp:r0 + dp + ROWS, dq:dq + W]
        patf = pat.reshape(9, A, NPOS)
        ph = patf.astype(f16).astype(np.float32)
        pl = (patf - ph).astype(f16)
        pstack = np.empty((K27, A, NPOS), dtype=f16)
        pstack[0:9] = ph.astype(f16)
        pstack[9:18] = pl
        pstack[18:27] = ph.astype(f16)

        pall = np.zeros((NJ * 128, NPOS), dtype=f16)
        wall = np.zeros((NJ * 128, CB), dtype=f16)
        pall[:A * K27] = pstack.transpose(1, 0, 2).reshape(A * K27, NPOS)
        wall[:A * K27] = wstack.transpose(1, 0, 2).reshape(A * K27, CB)
        per_core.append({
            "pstack": np.ascontiguousarray(pstack),
            "wstack": np.ascontiguousarray(wstack),
            "p1p": np.ascontiguousarray(
                pall.reshape(NJ, 128, NPOS).transpose(1, 0, 2)),
            "p1w": np.ascontiguousarray(
                wall.reshape(NJ, 128, CB).transpose(1, 0, 2)),
        })
    return per_core


def kernel(x=None, weights=None, num_routes=3, **kw):
    x = np.asarray(x, dtype=np.float32)
    weights = np.asarray(weights, dtype=np.float32)
    nr = int(num_routes)

    if nr not in _CACHE:
        _CACHE[nr] = _build_nc(nr)
    nc = _CACHE[nr]

    in_maps = _prep_inputs(x, weights)
    res = bass_utils.run_bass_kernel_spmd(nc, in_maps, core_ids=list(range(NCORES)))

    out = np.empty((B, C, H, W), dtype=np.float32)
    for core in range(NCORES):
        o = np.asarray(res.results[core]["out"]).astype(np.float32)
        o = o.reshape(ROWS, W, C, B)
        out[:, :, core * ROWS:(core + 1) * ROWS, :] = o.transpose(3, 2, 0, 1)
    return out


def profile_once(inputs):
    """Cost-model exec time via CoreSim (NTFF unavailable in this env)."""
    nr = int(inputs.get("num_routes", 3))
    if nr not in _CACHE:
        _CACHE[nr] = _build_nc(nr)
    nc = _CACHE[nr]
    from concourse import bass_interp
    sim = bass_interp.CoreSim(nc, no_exec=True, ignore_data_errors=True,
                              publish_trace=False)
    sim.simulate()
    return int(sim.time)
